# revision 10
# baseline (speedup 1.0000x reference)
"""PhiSoftMax (Gumbel-sigmoid masked attention) Trainium2 Bass kernel.

Shapes (hardcoded): B=4, L=S=1024, H=8, E=64.
Sharding: one head per NeuronCore (8 cores). Each core reads its head's
phi/u slice once and reuses it across all 4 batches -> minimal HBM traffic
(~28 MiB/core: phi 4 + u 4 + q/k/v 3 + att out 16 + V out 1).

Per-core math (head h), per batch b, per 128-row tile i of L:
  psum  = q_b^T k_b  (fp32r matmuls, contraction E=64 on partitions)
        + PM         (identity-weight bf16 matmuls; PM = MASK_SCALE*min(sigma, th)
                      [+ causal -BIG], split hi/lo bf16 for fp32-level accuracy)
  e     = Exp(SCALE*psum - SCALE*MASK_SCALE*th)   [ACT, accumulates Z rowwise]
  Sey   = sum(e * SCALE * psum)                   [DVE scalar_tensor_tensor accum]
  att   = e * (1/Z)                               [DVE tensor_scalar, AP scalar]
  attT  = PE transposes of bf16(att) -> PSUM -> SBUF staging
  V^T  += v_s^T @ attT_s                          [bf16 matmuls, fp32 PSUM accum]
  ent   = ln(Z) - Sey/Z + SCALE*MASK_SCALE*th
where sigma = Sigmoid((ln(u+eps) - ln(1-u+eps) + phi)/tau).

The soft-mask identity used: phi_mask = -MS*relu(th-sigma) = MS*min(sigma,th) - MS*th.
The +MS*min(sigma,th) part is added via PE; the constant -MS*th goes into the
ACT exp bias, so no max-subtraction pass is needed (scores are bounded ~5.5).
"""

import os
import numpy as np
import ml_dtypes

import concourse.bacc as bacc
import concourse.tile as tile
import concourse.mybir as mybir
from concourse.bass_utils import run_bass_kernel_spmd

B, L, S, H, E = 4, 1024, 1024, 8, 64
N_CORES = 8
MASK_SCALE = 10000.0
TAU_MIN, TAU_MAX = 0.1, 5.0
TH_MIN, TH_MAX = 0.01, 0.99
EPS = 1e-8
SCALE = 0.125  # 1/sqrt(E)
BIG = 262144.0  # causal mask magnitude; exact in bf16, SCALE*BIG >> 88

F32 = mybir.dt.float32
F32R = mybir.dt.float32r
BF16 = mybir.dt.bfloat16
AF = mybir.ActivationFunctionType
ALU = mybir.AluOpType

NT = L // 128  # 8 row tiles

_build_cache = {}


def _build(causal: bool, tau: float, thresh: float, repeat: int = 1):
    key = (causal, float(tau), float(thresh), repeat)
    if key in _build_cache:
        return _build_cache[key]

    nc = bacc.Bacc("TRN2", target_bir_lowering=False, debug=False)

    exp_bias = -SCALE * MASK_SCALE * thresh

    def _reg_const(value, dtype=F32):
        if (dtype, value) in nc.const_aps.aps:
            return
        t = nc.alloc_sbuf_tensor(f"const-{dtype.name}-{value}", [128, 1], dtype)
        nc.gpsimd.memset(t.ap(), value)
        nc.const_aps.aps[(dtype, value)] = t.ap()

    _reg_const(EPS)
    _reg_const(1.0 + EPS)
    nc.all_engine_barrier()

    qT_d = nc.dram_tensor("qT", [B, E, L], F32R, kind="ExternalInput").ap()
    kT_d = nc.dram_tensor("kT", [B, E, S], F32R, kind="ExternalInput").ap()
    v_d = nc.dram_tensor("v", [B, S, E], F32, kind="ExternalInput").ap()
    phi_d = nc.dram_tensor("phi", [L, S], F32, kind="ExternalInput").ap()
    u_d = nc.dram_tensor("u", [L, S], F32, kind="ExternalInput").ap()
    id_d = nc.dram_tensor("ident", [128, 128], BF16, kind="ExternalInput").ap()
    mst_d = nc.dram_tensor("master", [128, 896], F32, kind="ExternalInput").ap()

    att_d = nc.dram_tensor("att", [B, L, S], F32, kind="ExternalOutput").ap()
    vt_d = nc.dram_tensor("vt", [B, E, L], F32, kind="ExternalOutput").ap()
    ent_d = nc.dram_tensor("ent", [B, NT, 128], F32, kind="ExternalOutput").ap()

    def jn_of(i):
        return (i // 4 + 1) if causal else (S // 512)

    def nsb_of(i):
        return (i + 1) if causal else NT

    with tile.TileContext(nc) as tc:
        with (
            tc.tile_pool(name="consts", bufs=1) as consts,
            tc.tile_pool(name="pmpool", bufs=1) as pmpool,
        ):
            ident = consts.tile([128, 128], BF16)
            nc.sync.dma_start(ident[:], id_d[:])
            if causal:
                master = consts.tile([128, 896], F32)
                nc.sync.dma_start(master[:], mst_d[:])
                zerot = consts.tile([128, 512], F32)
                nc.vector.memset(zerot[:], 0.0)

            pm_hi = pmpool.tile([128, NT * 1024], BF16)
            pm_lo = pmpool.tile([128, NT * 1024], BF16)
            if causal:
                pmd_hi = pmpool.tile([128, NT * 512], BF16)
                pmd_lo = pmpool.tile([128, NT * 512], BF16)
            # Per-row exp bias BV[:, i] = exp_bias - SCALE*max_valid(pm_row):
            # keeps exp() in range even when the whole row is phi-masked
            # (the qk part is bounded by ~±6 after SCALE).
            BV = pmpool.tile([128, NT], F32)

            for rep in range(repeat):
                # ---------------- phi-mask phase ----------------
                with (
                    tc.tile_pool(name="gp_pool", bufs=1) as gp_pool,
                    tc.tile_pool(name="pmwork", bufs=2) as pw,
                ):
                    gp_all = gp_pool.tile([128, NT * 1024], F32)
                    # A: all Ln passes (one ACT table set)
                    for i in range(NT):
                        rows = slice(i * 128, (i + 1) * 128)
                        ut = pw.tile([128, 1024], F32, tag="ut")
                        nc.sync.dma_start(ut[:], u_d[rows, :])
                        ft = pw.tile([128, 1024], F32, tag="ft")
                        nc.sync.dma_start(ft[:], phi_d[rows, :])
                        t1 = pw.tile([128, 1024], F32, tag="t1")
                        nc.scalar.activation(t1[:], ut[:], AF.Ln, bias=EPS, scale=1.0)
                        t2 = pw.tile([128, 1024], F32, tag="t2")
                        nc.scalar.activation(t2[:], ut[:], AF.Ln, bias=1.0 + EPS,
                                             scale=-1.0)
                        g = pw.tile([128, 1024], F32, tag="g")
                        nc.gpsimd.tensor_tensor(g[:], t1[:], t2[:], ALU.subtract)
                        nc.vector.tensor_tensor(
                            gp_all[:, i * 1024:(i + 1) * 1024], g[:], ft[:], ALU.add)
                    # B: all Sigmoid passes, then min/mult + hi/lo splits
                    for i in range(NT):
                        isl = slice(i * 1024, (i + 1) * 1024)
                        sg = pw.tile([128, 1024], F32, tag="sg")
                        nc.scalar.activation(sg[:], gp_all[:, isl], AF.Sigmoid,
                                             scale=1.0 / tau)
                        pmf = pw.tile([128, 1024], F32, tag="pmf")
                        nc.vector.tensor_scalar(pmf[:], sg[:], thresh, MASK_SCALE,
                                                ALU.min, ALU.mult)
                        nc.gpsimd.tensor_copy(pm_hi[:, isl], pmf[:])
                        nc.vector.tensor_tensor(pm_lo[:, isl], pmf[:], pm_hi[:, isl],
                                                ALU.subtract)
                        pmx = pw.tile([128, 1], F32, tag="pmx")
                        if causal:
                            j = i // 4
                            s = 128 * (i % 4)
                            dsl = slice(i * 512, (i + 1) * 512)
                            pmdf = pw.tile([128, 512], F32, tag="pmdf")
                            nc.gpsimd.tensor_tensor(
                                pmdf[:], pmf[:, j * 512:(j + 1) * 512],
                                master[:, 384 - s:384 - s + 512], ALU.add)
                            nc.gpsimd.tensor_copy(pmd_hi[:, dsl], pmdf[:])
                            nc.vector.tensor_tensor(pmd_lo[:, dsl], pmdf[:],
                                                    pmd_hi[:, dsl], ALU.subtract)
                            # row max of valid pm: non-diag tiles + diag tile
                            nc.vector.tensor_reduce(
                                pmx[:], pmdf[:], mybir.AxisListType.X, ALU.max)
                            if j > 0:
                                pmx2 = pw.tile([128, 1], F32, tag="pmx2")
                                nc.vector.tensor_reduce(
                                    pmx2[:], pmf[:, :j * 512],
                                    mybir.AxisListType.X, ALU.max)
                                nc.vector.tensor_tensor(pmx[:], pmx[:], pmx2[:],
                                                        ALU.max)
                        else:
                            nc.vector.tensor_reduce(
                                pmx[:], pmf[:], mybir.AxisListType.X, ALU.max)
                        nc.vector.tensor_scalar(BV[:, i:i + 1], pmx[:], -SCALE,
                                                None, ALU.mult)

                # ---------------- main attention loop ----------------
                with (
                    tc.tile_pool(name="qkpool", bufs=2) as qk,
                    tc.tile_pool(name="vpool", bufs=2) as vp,
                    tc.tile_pool(name="epool", bufs=2) as ep,
                    tc.tile_pool(name="trpool", bufs=2) as trp,
                    tc.tile_pool(name="stats", bufs=2) as st,
                    tc.tile_pool(name="psy", bufs=2, space="PSUM") as psy,
                    tc.tile_pool(name="pst", bufs=2, space="PSUM") as pst,
                    tc.tile_pool(name="psv", bufs=1, space="PSUM") as psv,
                ):
                    for b in range(B):
                        qt = qk.tile([E, L], F32R, tag="qt")
                        nc.sync.dma_start(qt[:], qT_d[b])
                        kt = qk.tile([E, S], F32R, tag="kt")
                        nc.sync.dma_start(kt[:], kT_d[b])
                        vf = vp.tile([128, (S // 128) * E, ], F32, tag="vf")
                        nc.sync.dma_start(
                            vf[:], v_d[b].rearrange("(s p) e -> p s e", p=128))
                        vb = vp.tile([128, (S // 128) * E], BF16, tag="vb")
                        nc.gpsimd.tensor_copy(vb[:], vf[:])

                        Zb = st.tile([128, NT], F32, tag="Zb")
                        Sb = st.tile([128, NT], F32, tag="Sb")
                        rzb = st.tile([128, NT], F32, tag="rzb")
                        attTst = trp.tile([128, NT * 1024], BF16, tag="attTst")

                        for i in range(NT):
                            jn = jn_of(i)
                            W = 512 * jn
                            nsb = nsb_of(i)
                            rows = slice(i * 128, (i + 1) * 128)
                            ps = psy.tile([128, 1024], F32, tag="ps")
                            for j in range(jn):
                                jsl = slice(j * 512, (j + 1) * 512)
                                nc.tensor.matmul(
                                    ps[:, jsl], qt[:, rows], kt[:, jsl],
                                    start=True, stop=False)
                            for j in range(jn):
                                jsl = slice(j * 512, (j + 1) * 512)
                                diag = causal and j == i // 4
                                if diag:
                                    hi = pmd_hi[:, i * 512:(i + 1) * 512]
                                    lo = pmd_lo[:, i * 512:(i + 1) * 512]
                                else:
                                    hi = pm_hi[:, i * 1024 + j * 512:
                                               i * 1024 + (j + 1) * 512]
                                    lo = pm_lo[:, i * 1024 + j * 512:
                                               i * 1024 + (j + 1) * 512]
                                nc.tensor.matmul(ps[:, jsl], ident[:], hi,
                                                 start=False, stop=False)
                                nc.tensor.matmul(ps[:, jsl], ident[:], lo,
                                                 start=False, stop=True)

                            e = ep.tile([128, 1024], F32, tag="e")
                            nc.scalar.activation(
                                e[:, :W], ps[:, :W], AF.Exp, scale=SCALE,
                                bias=BV[:, i:i + 1], accum_out=Zb[:, i:i + 1])
                            scr = ep.tile([128, 1024], F32, tag="scr")
                            nc.vector.scalar_tensor_tensor(
                                out=scr[:, :W], in0=e[:, :W], scalar=SCALE,
                                in1=ps[:, :W], op0=ALU.mult, op1=ALU.mult,
                                accum_out=Sb[:, i:i + 1])
                            nc.vector.reciprocal(rzb[:, i:i + 1], Zb[:, i:i + 1])
                            att = ep.tile([128, 1024], F32, tag="att")
                            nc.vector.tensor_scalar(
                                att[:, :W], e[:, :W], rzb[:, i:i + 1], None,
                                ALU.mult)
                            nc.sync.dma_start(att_d[b, rows, :W], att[:, :W])
                            if causal and W < S:
                                nc.sync.dma_start(att_d[b, rows, W:],
                                                  zerot[:, :S - W])

                            attb = ep.tile([128, 1024], BF16, tag="attb")
                            nc.gpsimd.tensor_copy(attb[:, :W], att[:, :W])
                            tp = pst.tile([128, 1024], BF16, tag="tp")
                            for sblk in range(nsb):
                                ssl = slice(sblk * 128, (sblk + 1) * 128)
                                nc.tensor.transpose(tp[:, ssl], attb[:, ssl],
                                                    ident[:])
                            nc.vector.tensor_copy(
                                attTst[:, i * 1024:i * 1024 + nsb * 128],
                                tp[:, :nsb * 128])

                        # AV: V^T[e, l] accumulated per 512-wide l half
                        vps = psv.tile([E, L], F32, tag="vps")
                        attT3 = attTst[:].rearrange("p (i d) -> p i d", d=1024)
                        for half in range(L // 512):
                            smax = NT  # sigma blocks contributing
                            first = True
                            for sblk in range(smax):
                                i_lo = max(sblk, 4 * half) if causal else 4 * half
                                i_hi = 4 * half + 4
                                if i_lo >= i_hi:
                                    continue
                                rhs = attT3[:, i_lo:i_hi,
                                            sblk * 128:(sblk + 1) * 128]
                                out = vps[:, i_lo * 128:i_hi * 128]
                                last = (sblk == smax - 1)
                                nc.tensor.matmul(
                                    out, vb[:, sblk * E:(sblk + 1) * E], rhs,
                                    start=first, stop=last,
                                    skip_group_check=True)
                                first = False
                        vt_sb = st.tile([E, L], F32, tag="vt_sb")
                        nc.scalar.activation(vt_sb[:], vps[:], AF.Copy)
                        nc.sync.dma_start(vt_d[b], vt_sb[:])

                        # entropy = ln(Z) - Sey/Z + SCALE*MASK_SCALE*th
                        lnz = st.tile([128, NT], F32, tag="lnz")
                        nc.scalar.activation(lnz[:], Zb[:], AF.Ln)
                        ts_ = st.tile([128, NT], F32, tag="ts_")
                        nc.vector.tensor_tensor(ts_[:], Sb[:], rzb[:], ALU.mult)
                        entb = st.tile([128, NT], F32, tag="entb")
                        nc.vector.tensor_tensor(entb[:], lnz[:], ts_[:],
                                                ALU.subtract)
                        entc = st.tile([128, NT], F32, tag="entc")
                        nc.vector.tensor_tensor(entc[:], entb[:], BV[:],
                                                ALU.subtract)
                        nc.sync.dma_start(ent_d[b].rearrange("t p -> p t"),
                                          entc[:])

    nc.compile()
    _build_cache[key] = nc
    return nc


def _scalar(x):
    return float(np.asarray(x).reshape(-1)[0])


def _make_master():
    r = np.arange(128, dtype=np.int64)[:, None]
    mm = np.arange(896, dtype=np.int64)[None, :]
    return np.where(mm - 384 <= r, 0.0, -BIG).astype(np.float32)


def _reference_numpy(query, key, value, pos, phi, u, log_tau, threshold,
                     causal_mask):
    """Pure-numpy fallback replicating the reference (used only if input
    assumptions are violated, e.g. non-monotone pos)."""
    scale = np.float32(1.0 / np.sqrt(query.shape[-1]))
    scores = np.einsum("blhe,bshe->bhls", query, key).astype(np.float32)
    if causal_mask:
        p = pos[..., 0]
        M = np.where(p[:, None, :] > p[:, :, None], -np.inf, 0.0)
        scores = scores + M[:, None, :, :].astype(np.float32)
    tau = np.clip(np.exp(log_tau), TAU_MIN, TAU_MAX)
    gumbel = np.log(u + EPS) - np.log(1.0 - u + EPS)
    m_relaxed = 1.0 / (1.0 + np.exp(-((gumbel + phi) / tau)))
    th = np.clip(threshold, TH_MIN, TH_MAX)
    phi_mask = -MASK_SCALE * np.maximum(th - m_relaxed, 0.0)
    scores = scores + phi_mask[None]
    y = scale * scores
    y = y - np.max(y, axis=-1, keepdims=True)
    e = np.exp(y)
    att = e / e.sum(-1, keepdims=True)
    logp = np.log(np.clip(att, EPS, None))
    entropy = -np.sum(att * logp, axis=-1)
    entropy = np.nan_to_num(entropy, nan=0.0)
    V = np.einsum("bhls,bshd->blhd", att, value).astype(np.float32)
    return V, att.astype(np.float32), entropy.astype(np.float32)


def kernel(query, key, value, pos, phi, u, log_tau, threshold, causal_mask):
    query = np.asarray(query, dtype=np.float32)
    key = np.asarray(key, dtype=np.float32)
    value = np.asarray(value, dtype=np.float32)
    pos = np.asarray(pos, dtype=np.float32)
    phi = np.asarray(phi, dtype=np.float32)
    u = np.asarray(u, dtype=np.float32)
    causal = int(np.asarray(causal_mask)) != 0

    # The device kernel assumes the causal mask reduces to j > i (monotone
    # strictly-increasing positions, as produced by setup_inputs). Fall back
    # to numpy for anything else.
    shapes_ok = (query.shape == (B, L, H, E) and key.shape == (B, S, H, E)
                 and value.shape == (B, S, H, E) and phi.shape == (H, L, S)
                 and u.shape == (H, L, S))
    pos_ok = True
    if causal:
        p = pos[..., 0]
        pos_ok = bool(np.all(np.diff(p, axis=1) > 0)) and p.shape == (B, L)
    if not (shapes_ok and pos_ok):
        return _reference_numpy(query, key, value, pos, phi, u, log_tau,
                                threshold, causal_mask)

    lt = np.float32(_scalar(log_tau))
    th = np.float32(_scalar(threshold))
    tau = float(np.clip(np.exp(lt), np.float32(TAU_MIN), np.float32(TAU_MAX)))
    thresh = float(np.clip(th, np.float32(TH_MIN), np.float32(TH_MAX)))

    repeat = int(os.environ.get("KERNEL_REPEAT", "1"))
    nc = _build(causal, tau, thresh, repeat)

    ident = np.eye(128, dtype=ml_dtypes.bfloat16)
    master = _make_master()

    in_maps = []
    for h in range(N_CORES):
        qT_h = np.ascontiguousarray(query[:, :, h, :].transpose(0, 2, 1))
        kT_h = np.ascontiguousarray(key[:, :, h, :].transpose(0, 2, 1))
        v_h = np.ascontiguousarray(value[:, :, h, :])
        in_maps.append({
            "qT": qT_h, "kT": kT_h, "v": v_h,
            "phi": np.ascontiguousarray(phi[h]),
            "u": np.ascontiguousarray(u[h]),
            "ident": ident, "master": master,
        })

    res = run_bass_kernel_spmd(nc, in_maps, list(range(N_CORES)))

    att = np.empty((B, H, L, S), dtype=np.float32)
    V = np.empty((B, L, H, E), dtype=np.float32)
    entropy = np.empty((B, H, L), dtype=np.float32)
    for h in range(N_CORES):
        r = res.results[h]
        att[:, h] = r["att"]
        V[:, :, h, :] = r["vt"].transpose(0, 2, 1)
        entropy[:, h, :] = r["ent"].reshape(B, L)
    return V, att, entropy


if __name__ == "__main__":
    # quick self-driven run with random inputs
    rng = np.random.default_rng(0)
    inputs = dict(
        query=rng.standard_normal((B, L, H, E), dtype=np.float32),
        key=rng.standard_normal((B, S, H, E), dtype=np.float32),
        value=rng.standard_normal((B, S, H, E), dtype=np.float32),
        pos=np.arange(B * L, dtype=np.float32).reshape(B, L, 1),
        phi=rng.standard_normal((H, L, S), dtype=np.float32),
        u=rng.random((H, L, S), dtype=np.float32),
        log_tau=np.float32(np.log(2.0)),
        threshold=np.float32(0.5),
        causal_mask=1,
    )
    V, att, ent = kernel(**inputs)
    print("V", V.shape, "att", att.shape, "ent", ent.shape)
